# revision 53
# baseline (speedup 1.0000x reference)
"""AGNNConv (DGL-style) forward on 8 Trainium2 NeuronCores.

Reference semantics:
    Hn = H / max(||H||_2, eps)
    s_e = Hn[row_e] . Hn[col_e]          (cosine score per edge)
    attn = segment_softmax(s, row)       (softmax per destination)
    out[r] = sum_e attn_e * H[col_e]

|s_e| <= 1, so segment_max can be the constant 1.0 (softmax is
shift-invariant): w_e = exp(s_e - 1); out[r] = sum w_e H[col_e] / sum w_e.

Device mapping (one SPMD graph, 8 cores):
  * Destinations dealt to cores by global degree-sorted round robin.
  * The scalable gather (gpsimd dma_gather) takes int16 indices -> the node
    table is addressed in 4 col-windows of 25k rows.  Edges split into 4
    phases by col window; phases produce additive partials (T_b[d], W_b[d]).
  * Per phase, each core re-sorts its dsts by (dst-window, -phase-degree);
    blocks of 128 dsts sit one-per-partition, edges along the free axis at a
    uniform per-superblock capacity (tight after sorting).  Segment math is
    pure free-axis ops; pad slots are killed by a -1e30 additive mask before
    exp.
  * Phase 0 writes partial rows [T(64) | W(1) | junk(63)] sequentially to a
    per-core DRAM accumulator; phases 1-3 dma_scatter_add (DMA-datapath CCE
    f32 add) theirs onto it at int16 row ids (position in phase-0 order).
    A merge pass computes T / max(W, tiny) and the host inverts the phase-0
    permutation.
"""

import os
import sys
import numpy as np

sys.path.insert(0, "/opt/trn_rl_repo")

N_CORES = 8
P = 128
D = 64
NWIN = 4
WIN = 25000
WSTRIDE = WIN + 8        # table rows per window (last 8 are zero rows)
ZROW = WIN               # window-local index of the zero row
SLOT_BUDGET = 80     # max k*C edge slots per partition per superblock
KMAX = 24            # max 128-dst rows per superblock
FIX = 5              # DP: fixed cost (in cap-row units) per superblock


class NS:
    pass


# ----------------------------------------------------------------------------
# Host-side plan
# ----------------------------------------------------------------------------
def _plan(row, col, N):
    row = np.asarray(row, np.int64)
    col = np.asarray(col, np.int64)
    deg = np.bincount(row, minlength=N).astype(np.int64)

    # CSR with edges sorted by (row, col) so each row's edges are grouped by
    # col window
    order_e = np.lexsort((col, row))
    ecols = col[order_e].astype(np.int64)
    erows = row[order_e]
    starts = np.zeros(N + 1, np.int64)
    np.cumsum(deg, out=starts[1:])

    ewin = ecols // WIN
    deg_b = np.stack([np.bincount(erows[ewin == b], minlength=N)
                      for b in range(NWIN)])           # [NWIN, N]
    off_b = np.zeros((NWIN + 1, N), np.int64)
    np.cumsum(deg_b, axis=0, out=off_b[1:])

    # ---- deal nodes to cores (global degree sort, 128-chunk round robin) ---
    order0g = np.argsort(-deg, kind="stable")
    NR0 = -(-N // (P * N_CORES))
    Ntot = NR0 * P * N_CORES
    opad = np.full(Ntot, -1, np.int64)
    opad[:N] = order0g
    chunks = opad.reshape(NR0, N_CORES, P)
    core_nodes = [chunks[:, c, :].reshape(-1) for c in range(N_CORES)]

    plan = NS()
    plan.phases = []                       # per phase: list of superblocks
    plan.flat = [[None] * NWIN for _ in range(N_CORES)]  # dst order per phase

    # ---- per-phase per-core sorted orders with equalized window groups ----
    for b in range(NWIN):
        per_core = []
        for c in range(N_CORES):
            nodes = core_nodes[c]
            win = np.where(nodes >= 0, nodes // WIN, NWIN - 1)
            db = np.where(nodes >= 0, deg_b[b][np.clip(nodes, 0, N - 1)], -1)
            srt = np.lexsort((-db, win))
            per_core.append((nodes[srt], win[srt]))
        gsz = np.array([[int((w == g).sum()) for g in range(NWIN)]
                        for _, w in per_core])
        gpad = (-(-gsz.max(axis=0) // P)) * P            # equal across cores
        flats = []
        for c in range(N_CORES):
            nodes, w = per_core[c]
            parts = []
            for g in range(NWIN):
                grp = nodes[w == g]
                parts.append(np.concatenate(
                    [grp, np.full(gpad[g] - len(grp), -1, np.int64)]))
            flats.append(np.concatenate(parts))
        NAb = len(flats[0])
        # capacity per 128-row, max across cores (shared graph)
        nrows = NAb // P
        caps = np.zeros(nrows, np.int64)
        for c in range(N_CORES):
            fl = flats[c]
            dbv = np.where(fl >= 0, deg_b[b][np.clip(fl, 0, N - 1)], 0)
            caps = np.maximum(caps, dbv.reshape(nrows, P).max(axis=1))
        rwin = np.repeat(np.arange(NWIN), gpad // P)     # window of each row
        # DP superblock partition: same window, k*C <= budget, k <= KMAX;
        # cost = k*Cmax (slots) + FIX per group (instruction overhead)
        supers = []
        i0 = 0
        while i0 < nrows:
            g = int(rwin[i0])
            j0 = i0
            while j0 < nrows and rwin[j0] == g:
                j0 += 1
            cseg = [max(int(caps[t]), 0) for t in range(i0, j0)]
            n = len(cseg)
            best = [0.0] * (n + 1)
            cut = [0] * (n + 1)
            for j in range(1, n + 1):
                b_ = None
                for i in range(max(0, j - KMAX), j):
                    cmax = max(cseg[i:j])
                    if cmax > 0 and (j - i) * cmax > SLOT_BUDGET:
                        continue
                    cost = best[i] + (j - i) * cmax + FIX
                    if b_ is None or cost < b_:
                        b_ = cost
                        cut[j] = i
                best[j] = b_ if b_ is not None else best[j - 1] + FIX
            # recover cuts
            bounds = []
            j = n
            while j > 0:
                i = cut[j]
                bounds.append((i, j))
                j = i
            for (i, j) in reversed(bounds):
                sbn = NS()
                sbn.win, sbn.k, sbn.C, sbn.row0 = \
                    g, j - i, max(cseg[i:j]), i0 + i
                supers.append(sbn)
            i0 = j0
        plan.phases.append(supers)
        for c in range(N_CORES):
            plan.flat[c][b] = flats[c]
        if b == 0:
            plan.NA = NAb

    # position of node in its core's phase-0 order
    pos0 = np.full((N_CORES, N), -1, np.int64)
    for c in range(N_CORES):
        fl = plan.flat[c][0]
        v = fl >= 0
        pos0[c][fl[v]] = np.nonzero(v)[0]
    plan.pos0 = pos0

    # ---- build per-core device streams --------------------------------
    def wrap16(a):
        # dma_gather/scatter idx stream: position i -> idxs[i%16, i//16],
        # replicated to 128 partitions (8 Q7 cores x 16)
        n = a.size
        w = a.reshape(n // 16, 16).T.astype(np.int16)
        return np.tile(w, (8, 1))

    plan.core = []
    for c in range(N_CORES):
        g_parts, r_parts, s_parts, m_parts = [], [], [], []
        for b in range(NWIN):
            fl = plan.flat[c][b]
            for sb in plan.phases[b]:
                g, k, C, row0 = sb.win, sb.k, sb.C, sb.row0
                if C == 0:
                    continue
                nodes = fl[row0 * P:(row0 + k) * P].reshape(k, P)
                rl = np.where(nodes >= 0, nodes, g * WIN)
                r_parts.append(rl.T)          # [P, k] node ids, filled later
                gblk = np.full((k, C, P), ZROW, np.int64)
                npad = np.full((P, k), float(C), np.float32)
                for rr in range(k):
                    for pp in range(P):
                        d = nodes[rr, pp]
                        if d < 0:
                            continue
                        nb = int(deg_b[b][d])
                        npad[pp, rr] = C - nb
                        if nb == 0:
                            continue
                        st = starts[d] + off_b[b][d]
                        gblk[rr, :nb, pp] = ecols[st:st + nb] - b * WIN
                g_parts.append(wrap16(gblk.reshape(-1)))
                m_parts.append(npad)
                if b > 0:
                    srow = np.where(nodes >= 0,
                                    pos0[c][np.clip(nodes, 0, N - 1)],
                                    plan.NA + np.arange(P)[None, :]
                                    .repeat(k, 0))
                    s_parts.append(wrap16(srow.reshape(-1)))
        cc = NS()
        cc.g16 = (np.concatenate(g_parts, axis=1) if g_parts
                  else np.zeros((P, 8), np.int16))
        cc.rnodes = (np.concatenate(r_parts, axis=1) if r_parts
                     else np.zeros((P, 1), np.int64))
        cc.s16 = (np.concatenate(s_parts, axis=1) if s_parts
                  else np.zeros((P, 8), np.int16))
        cc.mb = (np.concatenate(m_parts, axis=1) if m_parts
                 else np.zeros((P, 8), np.float32))
        plan.core.append(cc)
    return plan


def _make_table(H, N):
    import ml_dtypes
    tab = np.zeros((NWIN * WSTRIDE, 128), ml_dtypes.bfloat16)
    tab[:, D] = 1.0
    for b in range(NWIN):
        hi = min((b + 1) * WIN, N)
        tab[b * WSTRIDE:b * WSTRIDE + (hi - b * WIN), :D] = \
            H[b * WIN:hi].astype(ml_dtypes.bfloat16)
    return tab


# ----------------------------------------------------------------------------
# Numpy mirror of the device program (consumes the packed device arrays)
# ----------------------------------------------------------------------------
def _unwrap16(w, n):
    return w[:16].T.reshape(-1)[:n].astype(np.int64)


def _numpy_core(H, plan, c):
    import ml_dtypes
    tab = _make_table(H, H.shape[0]).astype(np.float32)   # [NWIN*WSTRIDE,128]
    NA = plan.NA
    cc = plan.core[c]
    accum = np.zeros((NA + P, 128), np.float32)
    E1 = np.float32(np.exp(np.float32(-1.0)))
    go = ro = so = mo = 0
    for b in range(NWIN):
        for sb in plan.phases[b]:
            g, k, C, row0 = sb.win, sb.k, sb.C, sb.row0
            if C == 0:
                continue
            nG = k * C * P
            rn_ = cc.rnodes[:, ro:ro + k]                  # [P, k] node ids
            gidx = _unwrap16(cc.g16[:, go:go + nG // 16], nG)
            npad = cc.mb[:, mo:mo + k]                     # [P, k]
            ro += k
            go += nG // 16
            mo += k
            w_ = rn_ // WIN
            loc = rn_ - w_ * WIN
            R = tab[(loc + w_ * WSTRIDE).T.reshape(-1), :D].reshape(k, P, D)
            # position i = ch*128 + p, ch = rr*C + j  ->  [k, C, P, D]
            G = tab[gidx + b * WSTRIDE, :D].reshape(k, C, P, D)
            rnR = 1.0 / np.sqrt((R * R).sum(-1) + 1e-12)   # [k, P]
            rnC = 1.0 / np.sqrt((G * G).sum(-1) + 1e-12)   # [k, C, P]
            G65 = tab[gidx + b * WSTRIDE, :D + 1].reshape(k, C, P, D + 1)
            t = (G * R[:, None, :, :]).sum(-1)             # [k, C, P]
            s = t * rnC * rnR[:, None, :]
            w = np.exp(s - 1.0)
            TW = (G65 * w[..., None]).sum(1)               # [k, P, D+1]
            Tb = TW[..., :D]
            Wb = TW[..., D] - npad.T * E1                  # [k, P]
            if b == 0:
                rows = row0 * P + np.arange(k * P)
                accum[rows, :D] += Tb.reshape(k * P, D)
                accum[rows, D] += Wb.reshape(k * P)
            else:
                nS = k * P
                srow = _unwrap16(cc.s16[:, so:so + nS // 16], nS)
                so += nS // 16
                pay = np.zeros((k * P, 128), np.float32)
                pay[:, :D] = Tb.reshape(k * P, D)
                pay[:, D] = Wb.reshape(k * P)
                np.add.at(accum, srow, pay)
    T = accum[:NA, :D]
    W = np.maximum(accum[:NA, D], 1e-20)
    return T / W[:, None]


def _unshuffle(outs, plan, N):
    full = np.zeros((N, D), np.float32)
    for c in range(N_CORES):
        o = plan.flat[c][0]
        m = o >= 0
        full[o[m]] = outs[c][m]
    return full


# ----------------------------------------------------------------------------
# Bass graph
# ----------------------------------------------------------------------------
def _build_graph(plan, N, Wg, Wr, Ws, Wm):
    from concourse import bass, mybir, bacc, library_config
    import concourse.tile as tile

    nc = bacc.Bacc("TRN2", target_bir_lowering=False, debug=False,
                   num_devices=N_CORES)
    f32 = mybir.dt.float32
    bf16 = mybir.dt.bfloat16
    i16 = mybir.dt.int16
    AT = mybir.AluOpType
    ACT = mybir.ActivationFunctionType
    X = mybir.AxisListType.X
    NA = plan.NA
    RW = 128                       # table row width (bf16), 64 real + 64 pad

    T_ext = nc.dram_tensor("T", [NWIN * WSTRIDE, RW], bf16,
                           kind="ExternalInput")
    g_ext = nc.dram_tensor("g", [P, Wg], i16, kind="ExternalInput")
    r_ext = nc.dram_tensor("r", [P, Wr], bf16, kind="ExternalInput")
    s_ext = nc.dram_tensor("s", [P, Ws], i16, kind="ExternalInput")
    m_ext = nc.dram_tensor("m", [P, Wm], f32, kind="ExternalInput")
    accum = nc.dram_tensor("accum", [NA + P, 128], f32, kind="ExternalOutput")
    out_ext = nc.dram_tensor("out", [NA, D], f32, kind="ExternalOutput")

    with tile.TileContext(nc) as tc:
        with tc.tile_pool(name="cst", bufs=1) as cst, \
             tc.tile_pool(name="gp", bufs=4) as gp, \
             tc.tile_pool(name="sl", bufs=2) as slp, \
             tc.tile_pool(name="sq", bufs=2) as sqp, \
             tc.tile_pool(name="pr", bufs=3) as prp, \
             tc.tile_pool(name="pk", bufs=3) as pkp, \
             tc.tile_pool(name="sm", bufs=4) as smp:
            nc.gpsimd.load_library(library_config.mlp)
            negone = cst.tile([P, 1], f32, tag="negone")
            nc.gpsimd.memset(negone[:], -1.0)
            ztile = cst.tile([P, 1], f32, tag="ztile")
            nc.gpsimd.memset(ztile[:], 0.0)
            magic = cst.tile([P, 1], mybir.dt.int32, tag="magic")
            nc.gpsimd.memset(magic[:], 0x5f37599e)
            # e1n = -exp(-1) with the exact ACT LUT value, so pad slots cancel
            e1 = cst.tile([P, 1], f32, tag="e1")
            nc.scalar.activation(e1[:], ztile[:], ACT.Exp, bias=negone[:])
            e1n = cst.tile([P, 1], f32, tag="e1n")
            nc.vector.tensor_scalar_mul(out=e1n[:], in0=e1[:], scalar1=-1.0)

            dbg_nwin = int(os.environ.get("DBG_NWIN", NWIN))
            dbg_scatter = int(os.environ.get("DBG_SCATTER", 1))
            dbg_merge = int(os.environ.get("DBG_MERGE", 1))
            go = ro = so = mo = 0
            for b in range(NWIN):
                if b >= dbg_nwin:
                    break
                hwin = T_ext[b * WSTRIDE:(b + 1) * WSTRIDE, :]
                live = [sb for sb in plan.phases[b] if sb.C > 0]
                pw_g = sum(sb.k * sb.C * P // 16 for sb in live)
                pw_r = sum(sb.k * D for sb in live)
                pw_m = sum(sb.k for sb in live)
                gslab = slp.tile([P, max(pw_g, 8)], i16, tag="gslab")
                nc.sync.dma_start(out=gslab[:, :pw_g],
                                  in_=g_ext[:, go:go + pw_g])
                rslab = slp.tile([P, max(pw_r, 8)], bf16, tag="rslab")
                nc.sync.dma_start(out=rslab[:, :pw_r],
                                  in_=r_ext[:, ro:ro + pw_r])
                mslab = slp.tile([P, max(pw_m, 8)], f32, tag="mslab")
                nc.sync.dma_start(out=mslab[:, :pw_m],
                                  in_=m_ext[:, mo:mo + pw_m])
                if b > 0:
                    pw_s = sum(sb.k * P // 16 for sb in live)
                    sslab = slp.tile([P, max(pw_s, 8)], i16, tag="sslab")
                    nc.sync.dma_start(out=sslab[:, :pw_s],
                                      in_=s_ext[:, so:so + pw_s])
                    so += pw_s
                go += pw_g
                ro += pw_r
                mo += pw_m
                lgo = lro = lmo = lso = 0
                for sb in plan.phases[b]:
                    g, k, C, row0 = sb.win, sb.k, sb.C, sb.row0
                    if C == 0:
                        continue
                    nR = k * P
                    nG = k * C * P
                    kC = k * C
                    R3 = rslab[:, lro:lro + k * D].rearrange(
                        "p (k f) -> p k f", f=D)
                    gidx = gslab[:, lgo:lgo + nG // 16]
                    npad_t = mslab[:, lmo:lmo + k]
                    lro += k * D
                    lgo += nG // 16
                    lmo += k

                    G = gp.tile([P, kC * RW], bf16, tag="G")
                    nc.gpsimd.dma_gather(
                        out_ap=G[:].rearrange("p (c f) -> p c f", f=RW),
                        in_ap=hwin, idxs_ap=gidx,
                        num_idxs=nG, num_idxs_reg=nG, elem_size=RW,
                        single_packet=False)
                    G3 = G[:].rearrange("p (c f) -> p c f", f=RW)[:, :, 0:D]
                    G4 = G[:].rearrange("p (k c f) -> p k c f",
                                        k=k, f=RW)[:, :, :, 0:D]

                    # norms (device-side; zero rows give t=0 exactly)
                    nsqA = smp.tile([P, k + kC], f32, tag="nsqA")
                    sqR = smp.tile([P, k * D], bf16, tag="sqR")
                    nc.scalar.activation(
                        sqR[:].rearrange("p (k f) -> p k f", f=D),
                        R3, ACT.Square)
                    nc.vector.tensor_reduce(
                        out=nsqA[:, 0:k],
                        in_=sqR[:].rearrange("p (k f) -> p k f", f=D),
                        axis=X, op=AT.add)

                    sqG = sqp.tile([P, kC * D], bf16, tag="sqG")
                    nc.scalar.activation(
                        sqG[:].rearrange("p (c f) -> p c f", f=D),
                        G3, ACT.Square)
                    # depth-2 bf16 tree then reduce (DVE reduce is 1x-only)
                    sg3 = sqG[:].rearrange("p (c f) -> p c f", f=D)
                    th1 = sqp.tile([P, kC * D // 2], bf16, tag="th1")
                    t13 = th1[:].rearrange("p (c f) -> p c f", f=D // 2)
                    nc.vector.tensor_tensor(out=t13, in0=sg3[:, :, 0:D // 2],
                                            in1=sg3[:, :, D // 2:D],
                                            op=AT.add)
                    th2 = sqp.tile([P, kC * D // 4], bf16, tag="th2")
                    t23 = th2[:].rearrange("p (c f) -> p c f", f=D // 4)
                    nc.vector.tensor_tensor(out=t23, in0=t13[:, :, 0:D // 4],
                                            in1=t13[:, :, D // 4:D // 2],
                                            op=AT.add)
                    nc.vector.tensor_reduce(out=nsqA[:, k:],
                                            in_=t23, axis=X, op=AT.add)
                    # rsqrt(x) via int magic + one Newton step (DVE only)
                    kk = k + kC
                    sh = smp.tile([P, kk], mybir.dt.int32, tag="sh")
                    nc.vector.tensor_scalar(
                        out=sh[:], in0=nsqA[:].bitcast(mybir.dt.int32),
                        scalar1=1, scalar2=None,
                        op0=AT.logical_shift_right)
                    yi = smp.tile([P, kk], mybir.dt.int32, tag="yi")
                    nc.vector.scalar_tensor_tensor(
                        out=yi[:],
                        in0=magic[:].broadcast_to([P, kk]),
                        scalar=0.0, in1=sh[:],
                        op0=AT.bypass, op1=AT.subtract)
                    rnA = smp.tile([P, kk], f32, tag="rnA")
                    hx = smp.tile([P, kk], f32, tag="hx")
                    nc.vector.tensor_scalar_mul(out=hx[:], in0=nsqA[:],
                                                scalar1=-0.5)
                    y2 = smp.tile([P, kk], f32, tag="y2")
                    y = yi[:].bitcast(f32)
                    nc.vector.tensor_tensor(out=y2[:], in0=y, in1=y,
                                            op=AT.mult)
                    nc.vector.scalar_tensor_tensor(
                        out=y2[:], in0=y2[:], scalar=1.5,
                        in1=hx[:], op0=AT.bypass, op1=AT.mult)
                    nc.vector.tensor_scalar_add(out=y2[:], in0=y2[:],
                                                scalar1=1.5)
                    nc.vector.tensor_tensor(out=rnA[:], in0=y, in1=y2[:],
                                            op=AT.mult)
                    rnR = rnA[:, 0:k]
                    rnC = rnA[:, k:]

                    # dots (bf16 2x)
                    prod = prp.tile([P, kC * D], bf16, tag="prod")
                    nc.vector.tensor_tensor(
                        out=prod[:].rearrange("p (k c f) -> p k c f",
                                              k=k, f=D),
                        in0=G4,
                        in1=R3.rearrange("p k (o f) -> p k o f", o=1)
                            .broadcast_to([P, k, C, D]),
                        op=AT.mult)
                    pr3 = prod[:].rearrange("p (c f) -> p c f", f=D)
                    nc.vector.tensor_tensor(out=t13, in0=pr3[:, :, 0:D // 2],
                                            in1=pr3[:, :, D // 2:D],
                                            op=AT.add)
                    nc.vector.tensor_tensor(out=t23, in0=t13[:, :, 0:D // 4],
                                            in1=t13[:, :, D // 4:D // 2],
                                            op=AT.add)
                    t = smp.tile([P, kC], f32, tag="t")
                    nc.vector.tensor_reduce(out=t[:], in_=t23,
                                            axis=X, op=AT.add)

                    # scores
                    s_t = smp.tile([P, kC], f32, tag="s")
                    nc.vector.tensor_tensor(out=s_t[:], in0=t[:], in1=rnC,
                                            op=AT.mult)
                    nc.vector.tensor_tensor(
                        out=s_t[:], in0=s_t[:],
                        in1=rnR.rearrange("p (k o) -> p k o", o=1)
                            .broadcast_to([P, k, C]),
                        op=AT.mult)

                    # w = exp(s-1); pad slots contribute exactly exp(-1),
                    # cancelled via npad * e1n below
                    w_t = smp.tile([P, kC], f32, tag="w")
                    nc.scalar.activation(w_t[:], s_t[:], ACT.Exp,
                                         bias=negone[:])
                    wb = smp.tile([P, kC], bf16, tag="wb")
                    nc.vector.tensor_copy(wb[:], w_t[:])

                    packed = pkp.tile([P, k * 128], f32, tag="packed")
                    pk3 = packed[:].rearrange("p (k f) -> p k f", f=128)
                    if b > 0 and dbg_scatter:
                        nc.gpsimd.memset(pk3[:, :, D + 1:], 0.0)

                    # weighted sum over 66-wide rows (slot 64 is constant
                    # 1.0 from the table so reduce col 64 = sum(w); 66*2B
                    # keeps rows 4B-aligned for gpsimd)
                    D1 = D + 2
                    G4x = G[:].rearrange("p (k c f) -> p k c f",
                                         k=k, f=RW)[:, :, :, 0:D1]
                    prod66 = prp.tile([P, kC * D1], bf16, tag="prod66")
                    nc.gpsimd.tensor_tensor(
                        out=prod66[:].rearrange("p (k c f) -> p k c f",
                                                k=k, f=D1),
                        in0=G4x,
                        in1=wb[:].rearrange("p (c o) -> p c o", o=1)
                            .broadcast_to([P, kC, D1])
                            .rearrange("p (k c) f -> p k c f", k=k),
                        op=AT.mult)
                    if C % 2 == 0 and C >= 4:
                        # level-1 tree over C, then strided reduce
                        p4 = prod66[:].rearrange("p (k c f) -> p k c f",
                                                 k=k, f=D1)
                        tc1 = prp.tile([P, k * (C // 2) * D1], bf16,
                                       tag="tc1")
                        tc13 = tc1[:].rearrange("p (k c f) -> p k c f",
                                                k=k, f=D1)
                        nc.gpsimd.tensor_tensor(out=tc13,
                                                in0=p4[:, :, 0:C // 2, :],
                                                in1=p4[:, :, C // 2:C, :],
                                                op=AT.add)
                        nc.vector.tensor_reduce(
                            out=pk3[:, :, 0:D1],
                            in_=tc1[:].rearrange("p (k c f) -> p k f c",
                                                 k=k, f=D1),
                            axis=X, op=AT.add)
                    else:
                        nc.vector.tensor_reduce(
                            out=pk3[:, :, 0:D1],
                            in_=prod66[:].rearrange("p (k c f) -> p k f c",
                                                    k=k, f=D1),
                            axis=X, op=AT.add)
                    # cancel pad slots' exp(-1) contributions to W (col 64)
                    nc.vector.scalar_tensor_tensor(
                        out=pk3[:, :, D:D + 1],
                        in0=npad_t.rearrange("p (k o) -> p k o", o=1),
                        scalar=e1n[:],
                        in1=pk3[:, :, D:D + 1],
                        op0=AT.mult, op1=AT.add)

                    if b == 0:
                        nc.sync.dma_start(
                            out=accum[row0 * P:(row0 + k) * P, :D + 1]
                                .rearrange("(k p) f -> p k f", p=P),
                            in_=pk3[:, :, :D + 1])
                    elif dbg_scatter:
                        nS = k * P
                        sidx = sslab[:, lso:lso + nS // 16]
                        lso += nS // 16
                        nc.gpsimd.dma_scatter_add(
                            accum[:, :],
                            packed[:].rearrange("p (k f) -> p k f", f=128),
                            sidx,
                            nS, nS, 128, single_packet=False)

            # merge: out = T / max(W, tiny)
            MB = 8
            nblk = (NA // P) if dbg_merge else 0
            for m0 in range(0, nblk, MB):
                mb_n = min(MB, nblk - m0)
                acc_t = pkp.tile([P, MB * (D + 1)], f32, tag="acc")
                a3 = acc_t[:].rearrange("p (m f) -> p m f", f=D + 1)
                nc.sync.dma_start(
                    out=a3[:, :mb_n, :],
                    in_=accum[m0 * P:(m0 + mb_n) * P, :D + 1].rearrange(
                        "(m p) f -> p m f", p=P))
                Wg_t = smp.tile([P, MB], f32, tag="Wg")
                Wg3 = Wg_t[:].rearrange("p (m o) -> p m o", o=1)
                nc.vector.tensor_scalar_max(out=Wg3[:, :mb_n, :],
                                            in0=a3[:, :mb_n, D:D + 1],
                                            scalar1=1e-20)
                rW = smp.tile([P, MB], f32, tag="rW")
                nc.vector.reciprocal(rW[:, :mb_n], Wg_t[:, :mb_n])
                outb = smp.tile([P, MB * D], f32, tag="outb")
                o3 = outb[:].rearrange("p (m f) -> p m f", f=D)
                nc.vector.tensor_tensor(
                    out=o3[:, :mb_n, :],
                    in0=a3[:, :mb_n, 0:D],
                    in1=rW[:].rearrange("p (m o) -> p m o", o=1)
                        .broadcast_to([P, MB, D])[:, :mb_n, :],
                    op=AT.mult)
                nc.sync.dma_start(
                    out=out_ext[m0 * P:(m0 + mb_n) * P, :].rearrange(
                        "(m p) f -> p m f", p=P),
                    in_=o3[:, :mb_n, :])
    nc.compile()
    return nc


# ----------------------------------------------------------------------------
# Entry point
# ----------------------------------------------------------------------------
def kernel(H, row, col, num_nodes):
    H = np.asarray(H, np.float32)
    N = int(num_nodes)
    plan = _plan(np.asarray(row), np.asarray(col), N)

    if os.environ.get("KERNEL_NUMPY_ONLY"):
        outs = [_numpy_core(H, plan, c) for c in range(N_CORES)]
        return _unshuffle(outs, plan, N)

    from concourse.bass_utils import run_bass_kernel_spmd

    Wg = plan.core[0].g16.shape[1]
    Wr = plan.core[0].rnodes.shape[1] * D
    Ws = plan.core[0].s16.shape[1]
    Wm = plan.core[0].mb.shape[1]
    for c in range(1, N_CORES):
        assert plan.core[c].g16.shape[1] == Wg
        assert plan.core[c].s16.shape[1] == Ws
    nc = _build_graph(plan, N, Wg, Wr, Ws, Wm)
    tab = _make_table(H, N)

    def rbf(c):
        rn_ = plan.core[c].rnodes            # [P, Wr] node ids
        w_ = rn_ // WIN
        loc = rn_ - w_ * WIN
        rows = tab[(loc + w_ * WSTRIDE).reshape(-1), :D]
        return rows.reshape(P, -1, D).reshape(P, rn_.shape[1] * D)

    in_maps = [{"T": tab, "g": plan.core[c].g16, "r": rbf(c),
                "s": plan.core[c].s16, "m": plan.core[c].mb}
               for c in range(N_CORES)]
    trace = bool(os.environ.get("KERNEL_TRACE"))
    res = run_bass_kernel_spmd(nc, in_maps, core_ids=list(range(N_CORES)),
                               trace=trace)
    global LAST_RESULT, LAST_CTX
    LAST_RESULT = res
    LAST_CTX = (nc, in_maps[0])
    outs = [res.results[c]["out"] for c in range(N_CORES)]
    return _unshuffle(outs, plan, N)


LAST_RESULT = None
LAST_CTX = None
